# revision 33
# baseline (speedup 1.0000x reference)
"""Trainium2 Bass kernel for a 2-layer GAT (4 heads, 32 dim/head) + linear classifier.

Architecture (8 NeuronCores, SPMD; 3 device launches with host-side edge
expansion between them — the host only permutes/adds rows, all FLOPs run on
device):

- Host prep: append self-loops (plus one fake edge per padded node so every
  segment is non-empty and no eps/NaN handling is needed), sort edges by dst,
  partition dst nodes into 8 x 49 blocks of 128 nodes; uniform T tiles per
  block; edge j of a block maps to slot (p = j % 128, t = j // 128), so tile
  t holds 128 consecutive sorted edges and its dst-local values span a narrow
  band [lo_t, lo_t + W).

- Launch A (node transform): rows[n] = x[n] @ [W1 | W1@As1 | W1@Ad1] (bf16).

- Host expansion (per layer): EA[p, b*132T + c*T + t] holds, for the edge in
  slot (b,p,t), column c of [h[src] (128) | a_src[src]+a_dst[dst] (4)] in bf16,
  c-major so that (a) each partition's block chunk is one contiguous 132*T*2B
  DMA and (b) the per-edge broadcasts on device have packed last AP dims,
  unlocking the DVE 2x fast mode.

- Launch B/C (GAT layer): per block: one DMA; w = exp(lrelu(e)) computed as
  max(exp(e), exp(0.2e)) so ACT keeps a single function table; h *= w
  (head-expanded, split DVE/GPSIMD); one-hot lhsT: full 128-wide for tile 0
  (PSUM init), banded W-wide for tiles 1.. (vs dstloc - lo_t, with lo_t
  baked into the matmul partition offsets); T PE matmuls accumulate
  U = sum_t A01_t^T @ [h*w | w]_t in PSUM; h0 = U[:, :128] / s (single DVE
  divide); PE transpose; relu(hT + b_col) on DVE (scalar_tensor_tensor);
  PE matmul z = hT^T @ Wnext; z DMA'd straight from PSUM (f32; final bias
  bc is added on host).
"""

import os
import sys
import time

for _p in ("/opt/trn_rl_repo", "/root/.axon_site/_ro/trn_rl_repo"):
    if os.path.isdir(_p) and _p not in sys.path:
        sys.path.insert(0, _p)

import dataclasses

import numpy as np
import ml_dtypes

import concourse.bass as bass
import concourse.mybir as mybir
import concourse.tile as tile
from concourse import bacc
from concourse.bass_utils import run_bass_kernel_spmd

P = 128
D = 128
HEADS = 4
C = 40
NEG_SLOPE = 0.2
ROW = D + 2 * HEADS  # 136: [h | a_src | a_dst] (launch A / B outputs)
EAC = D + HEADS  # 132: [h | e] edge-expanded columns
NCORES = 8

f32 = mybir.dt.float32
bf16 = mybir.dt.bfloat16
nbf16 = ml_dtypes.bfloat16

LAST_INFO = {}  # timing info stash for test.py

# Engine-split tuning (Pool = GPSIMD; it cannot touch PSUM and its TT mult
# runs at ~1.98 ns/elem vs DVE 2x-mode 0.52, so DVE keeps PSUM ops + most of
# the h*w multiply while Pool takes the banded one-hot + the rest):
#   gw_dve_j: per-head h*w columns (of 32) multiplied on DVE
#   band_eng: engine for the banded one-hot ("gp" or "dve")
#   div: use a single TT divide (else reciprocal+mult)
# div=False: a TensorTensor may read only ONE non-scalar PSUM input, so the
# node phase is reciprocal(PSUM s)->SBUF then U * rcp.
# GPSIMD HW codegen only supports a few TT ops (mult/add proven); is_equal
# fails the engine check, so all one-hot builds stay on DVE.
CONFIG = {"gw_dve_j": 20, "band_eng": "dve", "a01f_eng": "dve", "div": False,
          "band_dve_i": 64, "gw_tsplit": 3}


def _ap_with(ap, dims):
    return dataclasses.replace(ap, ap=dims)


def build_node_transform(nper, wcols, trn_type="TRN2"):
    """Launch A: rows[n] = xTs[:, n].T @ wcat  for n in [0, nper), bf16."""
    nc = bacc.Bacc(trn_type, target_bir_lowering=False, debug=False, num_devices=NCORES)
    xts_d = nc.dram_tensor("xts", [P, nper], bf16, kind="ExternalInput")
    wcat_d = nc.dram_tensor("wcat", [D, wcols], bf16, kind="ExternalInput")
    out_d = nc.dram_tensor("nrows", [nper, wcols], bf16, kind="ExternalOutput")
    nt = nper // P
    with tile.TileContext(nc) as tc:
        with (
            tc.tile_pool(name="const", bufs=1) as cpool,
            tc.tile_pool(name="work", bufs=4) as pool,
            tc.tile_pool(name="psum", bufs=4, space="PSUM") as psum,
        ):
            wcat = cpool.tile([D, wcols], bf16, tag="wcat")
            nc.sync.dma_start(wcat[:], wcat_d[:])
            for i in range(nt):
                xt = pool.tile([P, P], bf16, tag="xt")
                nc.sync.dma_start(xt[:], xts_d[:, i * P : (i + 1) * P])
                hp = psum.tile([P, wcols], f32, tag="hp")
                nc.tensor.matmul(hp[:], lhsT=xt[:], rhs=wcat[:], start=True, stop=True)
                hf = pool.tile([P, wcols], bf16, tag="hf")
                nc.scalar.activation(hf[:], hp[:], mybir.ActivationFunctionType.Copy)
                nc.sync.dma_start(out_d[i * P : (i + 1) * P, :], hf[:])
    nc.compile()
    return nc


def build_gat_layer(nper, nblocks, T, W, los, his, wcols, is_last, repeat=1,
                    trn_type="TRN2", cfg=None):
    """Launch B/C: edge aggregation + node phase for one GAT layer.

    los/his: int arrays [nblocks, T] of band starts / max dstloc (baked into
    matmul partition offsets; his < 0 marks tiles empty on every core).
    Inputs: ea [P, nblocks*EAC*T] bf16 c-major edge rows, meta [P, nblocks*T]
    bf16 (dstlocal, 300 sentinel), losr [P, nblocks*T] bf16 (lo_t replicated),
    iotae [P, D*T] bf16 (iotae[p, d*T+t] = d), iotaw [P, W*max(T-1,1)] bf16
    (iotaw[p, i*(T-1)+t'] = i), wnext [D, wcols] bf16, bcol [P, 1] f32,
    ident [P, P] bf16.  Output: zrows [nper, wcols] f32 (no out bias).
    """
    cfg = dict(CONFIG if cfg is None else cfg)
    gw_dve_j = int(cfg.get("gw_dve_j", 32))
    band_eng = cfg.get("band_eng", "gp")
    a01f_eng = cfg.get("a01f_eng", "dve")
    use_div = bool(cfg.get("div", True))
    odt = f32 if is_last else bf16
    TB = max(T - 1, 1)

    nc = bacc.Bacc(trn_type, target_bir_lowering=False, debug=False, num_devices=NCORES)
    ea_d = nc.dram_tensor("ea", [P, nblocks * EAC * T], bf16, kind="ExternalInput")
    meta_d = nc.dram_tensor("meta", [P, nblocks * T], bf16, kind="ExternalInput")
    losr_d = nc.dram_tensor("losr", [P, nblocks * T], bf16, kind="ExternalInput")
    iotae_d = nc.dram_tensor("iotae", [P, D], bf16, kind="ExternalInput")
    iotaw_d = nc.dram_tensor("iotaw", [P, W * TB], bf16, kind="ExternalInput")
    wnext_d = nc.dram_tensor("wnext", [D, wcols], bf16, kind="ExternalInput")
    bcol_d = nc.dram_tensor("bcol", [P, 1], f32, kind="ExternalInput")
    ident_d = nc.dram_tensor("ident", [P, P], bf16, kind="ExternalInput")
    out_d = nc.dram_tensor("zrows", [nper, wcols], odt, kind="ExternalOutput")

    with tile.TileContext(nc) as tc:
        with (
            tc.tile_pool(name="const", bufs=1) as cpool,
            tc.tile_pool(name="work", bufs=int(cfg.get("wbufs", 4))) as pool,
            tc.tile_pool(name="gath", bufs=int(cfg.get("gbufs", 4))) as gpool,
            tc.tile_pool(name="a01p", bufs=int(cfg.get("abufs", 4))) as apool,
            tc.tile_pool(name="psum", bufs=2, space="PSUM") as psum,
            tc.tile_pool(name="psU", bufs=int(cfg.get("ubufs", 4)), space="PSUM") as psU,
        ):
            meta = cpool.tile([P, nblocks * T], bf16, tag="meta")
            nc.sync.dma_start(meta[:], meta_d[:])
            losr = cpool.tile([P, nblocks * T], bf16, tag="losr")
            nc.sync.dma_start(losr[:], losr_d[:])
            iotae = cpool.tile([P, D], bf16, tag="iotae")
            nc.sync.dma_start(iotae[:], iotae_d[:])
            iotaw = cpool.tile([P, W * TB], bf16, tag="iotaw")
            nc.sync.dma_start(iotaw[:], iotaw_d[:])
            wnext = cpool.tile([D, wcols], bf16, tag="wnext")
            nc.sync.dma_start(wnext[:], wnext_d[:])
            bcol = cpool.tile([P, 1], f32, tag="bcol")
            nc.sync.dma_start(bcol[:], bcol_d[:])
            ident = cpool.tile([P, P], bf16, tag="ident")
            nc.sync.dma_start(ident[:], ident_d[:])

            iotawv = iotaw[:].rearrange("p (i t) -> p i t", t=TB)

            for rep in range(repeat):
                if rep:
                    tc.strict_bb_all_engine_barrier()
                for b in range(nblocks):
                    mt = meta[:, b * T : (b + 1) * T]

                    # matmul plan: tile 0 full-width (PSUM init); banded tiles
                    # as 1-2 32-span matmuls (second only if band crosses)
                    plan = []  # (t, a, span)
                    for t in range(1, T):
                        if int(his[b, t]) < 0:
                            continue
                        a = int(los[b, t])
                        if W == P:
                            plan.append((t, 0, P))
                            continue
                        plan.append((t, a, 32))
                        if int(his[b, t]) >= a + 32:
                            plan.append((t, a + 32, 32))
                    nmm = 1 + len(plan)

                    # one-hot builds depend only on constants: emit them first
                    # so DVE/Pool run ahead while the edge DMA is in flight
                    a01f = apool.tile([P, D], bf16, tag="a01f")
                    in0f = _ap_with(mt[:, 0:1], [mt.ap[0], [0, D]])
                    in1f = _ap_with(iotae[:], [iotae[:].ap[0], [1, D]])
                    feng = nc.vector if a01f_eng == "dve" else nc.gpsimd
                    feng.tensor_tensor(out=a01f[:], in0=in0f, in1=in1f,
                                       op=mybir.AluOpType.is_equal)
                    if T > 1:
                        dl = pool.tile([P, TB], bf16, tag="dl")
                        beng = nc.gpsimd if band_eng == "gp" else nc.vector
                        beng.tensor_tensor(
                            out=dl[:], in0=mt[:, 1:T],
                            in1=losr[:, b * T + 1 : (b + 1) * T],
                            op=mybir.AluOpType.subtract,
                        )
                        AB = apool.tile([P, W * TB], bf16, tag="AB")
                        abv = AB[:].rearrange("p (i t) -> p i t", t=TB)
                        in0b = _ap_with(dl[:], [dl.ap[0], [0, W], dl.ap[1]])
                        bdi = int(cfg.get("band_dve_i", 16))
                        for eng, i0, i1 in (
                            (nc.vector, 0, bdi),
                            (nc.gpsimd, bdi, W),
                        ):
                            if i1 <= i0:
                                continue
                            in0s = _ap_with(
                                in0b, [in0b.ap[0], [0, i1 - i0], in0b.ap[2]]
                            )
                            eng.tensor_tensor(
                                out=abv[:, i0:i1, :], in0=in0s,
                                in1=iotawv[:, i0:i1, :],
                                op=mybir.AluOpType.is_equal,
                            )

                    G = gpool.tile([P, EAC * T], bf16, tag="G")
                    nc.sync.dma_start(G[:], ea_d[:, b * EAC * T : (b + 1) * EAC * T])

                    # w = exp(lrelu(e)); lrelu = (e*0.2) max e fused on DVE so ACT
                    # only ever runs Exp/Relu/Copy (one function table)
                    ecols = G[:, D * T : EAC * T]  # [P, 4*T]
                    if cfg.get("lrelu", "stt") == "stt":
                        nc.vector.scalar_tensor_tensor(
                            out=ecols, in0=ecols, scalar=float(NEG_SLOPE), in1=ecols,
                            op0=mybir.AluOpType.mult, op1=mybir.AluOpType.max,
                        )
                        nc.scalar.activation(ecols, ecols,
                                             mybir.ActivationFunctionType.Exp)
                    else:
                        w2 = pool.tile([P, HEADS * T], bf16, tag="w2")
                        nc.scalar.activation(w2[:], ecols,
                                             mybir.ActivationFunctionType.Exp,
                                             scale=NEG_SLOPE)
                        nc.scalar.activation(ecols, ecols,
                                             mybir.ActivationFunctionType.Exp)
                        nc.vector.tensor_tensor(out=ecols, in0=ecols, in1=w2[:],
                                                op=mybir.AluOpType.max)

                    # h *= w (head-expanded), in place, split by j-cols
                    gkjt = G[:, 0 : D * T].rearrange(
                        "p (k j t) -> p k j t", j=D // HEADS, t=T
                    )
                    wkt = G[:, D * T : EAC * T].rearrange("p (k t) -> p k t", t=T)
                    nsplit = max(1, int(cfg.get("gw_tsplit", 2)))
                    tcuts = [T * i // nsplit for i in range(nsplit + 1)]
                    for tl_, tr_ in zip(tcuts[:-1], tcuts[1:]):
                        for eng, j0, j1 in (
                            (nc.vector, 0, gw_dve_j),
                            (nc.gpsimd, gw_dve_j, D // HEADS),
                        ):
                            if j1 <= j0:
                                continue
                            o = gkjt[:, :, j0:j1, tl_:tr_]
                            wk = wkt[:, :, tl_:tr_]
                            wexp = _ap_with(
                                wk, [wk.ap[0], wk.ap[1], [0, j1 - j0], wk.ap[2]]
                            )
                            eng.tensor_tensor(out=o, in0=o, in1=wexp,
                                              op=mybir.AluOpType.mult)

                    g4 = G[:].rearrange("p (c t) -> p t c", t=T)
                    U = psU.tile([P, EAC], f32, tag="U")
                    nc.tensor.matmul(U[:], lhsT=a01f[:], rhs=g4[:, 0, :],
                                     start=True, stop=(nmm == 1),
                                     tile_position=(0, 0))
                    if T > 1:
                        ab4 = AB[:].rearrange("p (i t) -> p t i", t=TB)
                        for im, (t, a, span) in enumerate(plan):
                            lo0 = int(los[b, t])
                            i0 = a - lo0
                            nc.tensor.matmul(
                                U[a : a + span, :],
                                lhsT=ab4[:, t - 1, i0 : i0 + span],
                                rhs=g4[:, t, :], start=False,
                                stop=(im == len(plan) - 1),
                                skip_group_check=True,
                                tile_position=(0, a),
                            )

                    # node phase: h0 = U[:, :D] / s (pad-node fake edges keep s>0)
                    sexp = _ap_with(U[:, D:EAC], [U[:, D:EAC].ap[0], [1, HEADS],
                                                  [0, D // HEADS]])
                    h0 = pool.tile([P, D], bf16, tag="h0")
                    h0v = h0[:].rearrange("p (k j) -> p k j", j=D // HEADS)
                    uv = U[:, 0:D].rearrange("p (k j) -> p k j", j=D // HEADS)
                    if use_div:
                        nc.vector.tensor_tensor(out=h0v, in0=uv, in1=sexp,
                                                op=mybir.AluOpType.divide)
                    else:
                        rcp = pool.tile([P, HEADS], f32, tag="rcp")
                        nc.vector.reciprocal(rcp[:], U[:, D:EAC])
                        rexp = _ap_with(rcp[:], [rcp[:].ap[0], [1, HEADS],
                                                 [0, D // HEADS]])
                        nc.vector.tensor_tensor(out=h0v, in0=uv, in1=rexp,
                                                op=mybir.AluOpType.mult)

                    # transpose; relu(hT + b) fused on ACT (same table as Exp);
                    # project; z copied out on ACT (Copy, same table)
                    hTp = psum.tile([P, P], bf16, tag="hTp")
                    nc.tensor.transpose(hTp[:], h0[:], ident[:])
                    hT = pool.tile([P, P], bf16, tag="hT")
                    nc.scalar.activation(hT[:], hTp[:], mybir.ActivationFunctionType.Relu,
                                         bias=bcol[:])
                    zp = psum.tile([P, wcols], f32, tag="zp")
                    nc.tensor.matmul(zp[:], lhsT=hT[:], rhs=wnext[:], start=True, stop=True)
                    z = pool.tile([P, wcols], odt, tag="z")
                    nc.scalar.activation(z[:], zp[:], mybir.ActivationFunctionType.Copy)
                    nc.sync.dma_start(out_d[b * P : (b + 1) * P, :], z[:])

    nc.compile()
    return nc


def prep_edges(edge_index, n, ncores):
    """Sort self-looped edges (plus fake pad-node edges) by dst; uniform-T
    t-major slot layout with per-tile dst bands.

    Returns (T, W, nblocks, slots, srcs, dsts, metas, los, losr, npad):
    slots[c][e] = padded linear index b*128*T + (j%128)*T + (j//128);
    los [nblocks, T] int band starts shared across cores; losr its [P, nb*T]
    bf16 replication; metas[c] [P, nblocks*T] bf16 dstlocal (300 = empty).
    """
    nper = -(-n // (ncores * P)) * P
    npad = nper * ncores
    nblocks = nper // P

    e0 = np.asarray(edge_index[0], dtype=np.int64)
    e1 = np.asarray(edge_index[1], dtype=np.int64)
    loops = np.arange(n, dtype=np.int64)
    # fake edges for padded nodes: enough per node to keep tile dst-bands
    # narrow AND guarantee s > 0 (no eps / NaN handling on device)
    fake_per = max(1, int(round((len(e0) + n) / max(n, 1))))
    pads = np.repeat(np.arange(n, npad, dtype=np.int64), fake_per)
    src = np.concatenate([e0, loops, np.zeros(len(pads), np.int64)])
    dst = np.concatenate([e1, loops, pads])
    order = np.argsort(dst, kind="stable")
    srcs_all = src[order]
    dsts_all = dst[order]

    bounds = np.searchsorted(dsts_all, np.arange(0, npad + 1, P))
    counts = bounds[1:] - bounds[:-1]
    T = max(1, int(-(-counts.max() // P)))

    # band bounds per (core, block, tile)
    lo_s = np.full((nblocks, T), 128, dtype=np.int64)
    hi_s = np.full((nblocks, T), -1, dtype=np.int64)
    slots, srcs, dsts, metas = [], [], [], []
    for c in range(ncores):
        meta_pad = np.full(nblocks * P * T, 300.0, dtype=np.float32)
        sl_parts = []
        lo_c, hi_c = int(bounds[c * nblocks]), int(bounds[(c + 1) * nblocks])
        for b in range(nblocks):
            g = c * nblocks + b
            lo, hi = int(bounds[g]), int(bounds[g + 1])
            cnt = hi - lo
            j = np.arange(cnt)
            slot = b * P * T + (j % P) * T + (j // P)
            sl_parts.append(slot)
            dloc = (dsts_all[lo:hi] - g * P).astype(np.int64)
            meta_pad[slot] = dloc.astype(np.float32)
            ntile = -(-cnt // P)
            for t in range(1, ntile):  # tile 0 is always full-width
                seg = dloc[t * P : (t + 1) * P]
                lo_s[b, t] = min(lo_s[b, t], int(seg[0]))
                hi_s[b, t] = max(hi_s[b, t], int(seg[-1]))
        slots.append(np.concatenate(sl_parts))
        srcs.append(srcs_all[lo_c:hi_c])
        dsts.append(dsts_all[lo_c:hi_c])
        meta = (
            meta_pad.reshape(nblocks, P, T).transpose(1, 0, 2).reshape(P, nblocks * T)
        )
        metas.append(np.ascontiguousarray(meta.astype(nbf16)))

    # PE PSUM writes are power-of-2 tiled: span<=32 may start at 0/32/64/96.
    # Band = 64-wide window at a 32-aligned start (covers any raw band <= 33
    # wide); aggregation emits 1-2 32-span matmuls per tile.
    a_s = np.where(hi_s >= 0, np.minimum((lo_s // 32) * 32, P - P // 2), 0)
    if bool(((hi_s < a_s + P // 2) | (hi_s < 0)).all()):
        W = P // 2
    else:  # fallback: full-width one-hot for every tile
        W = P
        a_s = np.zeros_like(a_s)
    losr = np.ascontiguousarray(
        np.broadcast_to(
            a_s.reshape(1, nblocks * T).astype(np.float32), (P, nblocks * T)
        ).astype(nbf16)
    )
    return T, W, nblocks, slots, srcs, dsts, metas, a_s, hi_s, losr, npad


def build_ea(nrows_full, slots, srcs, dsts, nblocks, T):
    """EA[p, b*132T + c*T + t] = col c of [h[src] | a_src[src]+a_dst[dst]]."""
    eas = []
    for sl, ss, ds in zip(slots, srcs, dsts):
        vals = np.zeros((len(ss), EAC), dtype=nbf16)
        vals[:, 0:D] = nrows_full[ss, 0:D]
        vals[:, D:EAC] = (
            nrows_full[ss, D : D + HEADS].astype(np.float32)
            + nrows_full[ds, D + HEADS : ROW].astype(np.float32)
        ).astype(nbf16)
        pad = np.zeros((nblocks * P * T, EAC), dtype=nbf16)
        pad[sl] = vals
        ea = (
            pad.reshape(nblocks, P, T, EAC)
            .transpose(1, 0, 3, 2)
            .reshape(P, nblocks * EAC * T)
        )
        eas.append(np.ascontiguousarray(ea))
    return eas


def amat(att):
    A = np.zeros((D, HEADS), dtype=np.float32)
    att = np.asarray(att, dtype=np.float32)
    for h in range(HEADS):
        A[h * (D // HEADS) : (h + 1) * (D // HEADS), h] = att[h]
    return A


_cache = {}


def run_gat(x, edge_index, W1, att_src1, att_dst1, b1, W2, att_src2, att_dst2, b2,
            Wc, bc, n=None, ncores=NCORES, repeat=1):
    global LAST_INFO
    x = np.asarray(x, dtype=np.float32)
    if n is None:
        n = int(x.shape[0])

    t0 = time.time()
    T, W, nblocks, slots, srcs, dsts, metas, los, his, losr, npad = prep_edges(
        edge_index, n, ncores
    )
    nper = npad // ncores
    cfg = dict(CONFIG)
    key = (npad, T, W, nblocks, los.tobytes(), his.tobytes(), ncores, repeat,
           tuple(sorted(cfg.items())))
    t1 = time.time()
    if key in _cache:
        ncA, ncB, ncC = _cache[key]
    else:
        ncA = build_node_transform(nper, ROW)
        ncB = build_gat_layer(nper, nblocks, T, W, los, his, ROW, is_last=False,
                              repeat=repeat, cfg=cfg)
        ncC = build_gat_layer(nper, nblocks, T, W, los, his, C, is_last=True,
                              repeat=repeat, cfg=cfg)
        _cache[key] = (ncA, ncB, ncC)
    t2 = time.time()

    W1 = np.asarray(W1, dtype=np.float32)
    W2 = np.asarray(W2, dtype=np.float32)
    Wc = np.asarray(Wc, dtype=np.float32)
    w1cat = np.concatenate([W1, W1 @ amat(att_src1), W1 @ amat(att_dst1)], axis=1)
    w2cat = np.concatenate([W2, W2 @ amat(att_src2), W2 @ amat(att_dst2)], axis=1)
    b1c = np.asarray(b1, np.float32).reshape(P, 1)
    b2c = np.asarray(b2, np.float32).reshape(P, 1)
    bc = np.asarray(bc, dtype=np.float32)
    TB = max(T - 1, 1)
    iotae = np.ascontiguousarray(
        np.broadcast_to(np.arange(P, dtype=np.float32)[None, :], (P, P)).astype(nbf16)
    )
    iotaw = np.ascontiguousarray(
        np.broadcast_to(
            np.repeat(np.arange(W, dtype=np.float32), TB)[None, :], (P, W * TB)
        ).astype(nbf16)
    )
    ident = np.eye(P, dtype=nbf16)

    xp = np.zeros((npad, D), dtype=np.float32)
    xp[:n] = x

    # Launch A
    mapsA = [
        {
            "xts": np.ascontiguousarray(xp[c * nper : (c + 1) * nper].T).astype(nbf16),
            "wcat": w1cat.astype(nbf16),
        }
        for c in range(ncores)
    ]
    resA = run_bass_kernel_spmd(ncA, mapsA, list(range(ncores)))
    nrows_full = np.concatenate([resA.results[c]["nrows"] for c in range(ncores)], axis=0)
    t3 = time.time()

    # Expansion 1 + Launch B
    eas = build_ea(nrows_full, slots, srcs, dsts, nblocks, T)
    mapsB = [
        {
            "ea": eas[c], "meta": metas[c], "losr": losr, "iotae": iotae,
            "iotaw": iotaw, "wnext": w2cat.astype(nbf16), "bcol": b1c,
            "ident": ident,
        }
        for c in range(ncores)
    ]
    resB = run_bass_kernel_spmd(ncB, mapsB, list(range(ncores)))
    zrows_full = np.concatenate([resB.results[c]["zrows"] for c in range(ncores)], axis=0)
    t4 = time.time()

    # Expansion 2 + Launch C
    eas2 = build_ea(zrows_full, slots, srcs, dsts, nblocks, T)
    mapsC = [
        {
            "ea": eas2[c], "meta": metas[c], "losr": losr, "iotae": iotae,
            "iotaw": iotaw, "wnext": Wc.astype(nbf16), "bcol": b2c,
            "ident": ident,
        }
        for c in range(ncores)
    ]
    resC = run_bass_kernel_spmd(ncC, mapsC, list(range(ncores)))
    out = np.concatenate([resC.results[c]["zrows"] for c in range(ncores)], axis=0)[:n]
    out = out + bc[None, :]
    t5 = time.time()

    LAST_INFO = {
        "prep_s": t1 - t0, "build_s": t2 - t1, "launchA_s": t3 - t2,
        "launchB_s": t4 - t3, "launchC_s": t5 - t4,
        "ncs": (ncA, ncB, ncC),
        "maps": (mapsA, mapsB, mapsC),
        "dims": (T, W, nblocks, nper, los, his),
    }
    print(
        f"[kernel] prep={t1 - t0:.2f}s build={t2 - t1:.2f}s A={t3 - t2:.2f}s "
        f"B={t4 - t3:.2f}s C={t5 - t4:.2f}s T={T} W={W}",
        file=sys.stderr,
    )
    return out.astype(np.float32)


def kernel(x, edge_index, W1, att_src1, att_dst1, b1, W2, att_src2, att_dst2, b2, Wc, bc):
    return run_gat(x, edge_index, W1, att_src1, att_dst1, b1,
                   W2, att_src2, att_dst2, b2, Wc, bc)


# revision 35
# speedup vs baseline: 1.9902x; 1.9902x over previous
"""Trainium2 Bass kernel for a 2-layer GAT (4 heads, 32 dim/head) + linear classifier.

Architecture (8 NeuronCores, SPMD; 3 device launches with host-side edge
expansion between them — the host only permutes/adds rows, all FLOPs run on
device):

- Host prep: append self-loops (plus one fake edge per padded node so every
  segment is non-empty and no eps/NaN handling is needed), sort edges by dst,
  partition dst nodes into 8 x 49 blocks of 128 nodes; uniform T tiles per
  block; edge j of a block maps to slot (p = j % 128, t = j // 128), so tile
  t holds 128 consecutive sorted edges and its dst-local values span a narrow
  band [lo_t, lo_t + W).

- Launch A (node transform): rows[n] = x[n] @ [W1 | W1@As1 | W1@Ad1] (bf16).

- Host expansion (per layer): EA[p, b*132T + c*T + t] holds, for the edge in
  slot (b,p,t), column c of [h[src] (128) | a_src[src]+a_dst[dst] (4)] in bf16,
  c-major so that (a) each partition's block chunk is one contiguous 132*T*2B
  DMA and (b) the per-edge broadcasts on device have packed last AP dims,
  unlocking the DVE 2x fast mode.

- Launch B/C (GAT layer): per block: one DMA; w = exp(lrelu(e)) computed as
  max(exp(e), exp(0.2e)) so ACT keeps a single function table; h *= w
  (head-expanded, split DVE/GPSIMD); one-hot lhsT: full 128-wide for tile 0
  (PSUM init), banded W-wide for tiles 1.. (vs dstloc - lo_t, with lo_t
  baked into the matmul partition offsets); T PE matmuls accumulate
  U = sum_t A01_t^T @ [h*w | w]_t in PSUM; h0 = U[:, :128] / s (single DVE
  divide); PE transpose; relu(hT + b_col) on DVE (scalar_tensor_tensor);
  PE matmul z = hT^T @ Wnext; z DMA'd straight from PSUM (f32; final bias
  bc is added on host).
"""

import os
import sys
import time

for _p in ("/opt/trn_rl_repo", "/root/.axon_site/_ro/trn_rl_repo"):
    if os.path.isdir(_p) and _p not in sys.path:
        sys.path.insert(0, _p)

import dataclasses

import numpy as np
import ml_dtypes

import concourse.bass as bass
import concourse.mybir as mybir
import concourse.tile as tile
from concourse import bacc
from concourse.bass_utils import run_bass_kernel_spmd

P = 128
D = 128
HEADS = 4
C = 40
NEG_SLOPE = 0.2
ROW = D + 2 * HEADS  # 136: [h | a_src | a_dst] (launch A / B outputs)
EAC = D + HEADS  # 132: [h | e] edge-expanded columns
NCORES = 8

f32 = mybir.dt.float32
bf16 = mybir.dt.bfloat16
nbf16 = ml_dtypes.bfloat16

LAST_INFO = {}  # timing info stash for test.py

# Engine-split tuning (Pool = GPSIMD; it cannot touch PSUM and its TT mult
# runs at ~1.98 ns/elem vs DVE 2x-mode 0.52, so DVE keeps PSUM ops + most of
# the h*w multiply while Pool takes the banded one-hot + the rest):
#   gw_dve_j: per-head h*w columns (of 32) multiplied on DVE
#   band_eng: engine for the banded one-hot ("gp" or "dve")
#   div: use a single TT divide (else reciprocal+mult)
# div=False: a TensorTensor may read only ONE non-scalar PSUM input, so the
# node phase is reciprocal(PSUM s)->SBUF then U * rcp.
# GPSIMD HW codegen only supports a few TT ops (mult/add proven); is_equal
# fails the engine check, so all one-hot builds stay on DVE.
CONFIG = {"gw_dve_j": 20, "band_eng": "dve", "a01f_eng": "dve", "div": False,
          "band_dve_i": 64, "gw_tsplit": 3}


def _ap_with(ap, dims):
    return dataclasses.replace(ap, ap=dims)


def build_node_transform(nper, wcols, repeat=1, trn_type="TRN2"):
    """Launch A: rows[n] = xTs[:, n].T @ wcat  for n in [0, nper), bf16."""
    nc = bacc.Bacc(trn_type, target_bir_lowering=False, debug=False, num_devices=NCORES)
    xts_d = nc.dram_tensor("xts", [P, nper], bf16, kind="ExternalInput")
    wcat_d = nc.dram_tensor("wcat", [D, wcols], bf16, kind="ExternalInput")
    out_d = nc.dram_tensor("nrows", [nper, wcols], bf16, kind="ExternalOutput")
    nt = nper // P
    with tile.TileContext(nc) as tc:
        with (
            tc.tile_pool(name="const", bufs=1) as cpool,
            tc.tile_pool(name="work", bufs=4) as pool,
            tc.tile_pool(name="psum", bufs=4, space="PSUM") as psum,
        ):
            wcat = cpool.tile([D, wcols], bf16, tag="wcat")
            nc.sync.dma_start(wcat[:], wcat_d[:])
            for i in [i for rep in range(repeat) for i in range(nt)]:
                xt = pool.tile([P, P], bf16, tag="xt")
                nc.sync.dma_start(xt[:], xts_d[:, i * P : (i + 1) * P])
                hp = psum.tile([P, wcols], f32, tag="hp")
                nc.tensor.matmul(hp[:], lhsT=xt[:], rhs=wcat[:], start=True, stop=True)
                hf = pool.tile([P, wcols], bf16, tag="hf")
                nc.scalar.activation(hf[:], hp[:], mybir.ActivationFunctionType.Copy)
                nc.sync.dma_start(out_d[i * P : (i + 1) * P, :], hf[:])
    nc.compile()
    return nc


def build_gat_layer(nper, nblocks, T, W, los, his, wcols, is_last, repeat=1,
                    trn_type="TRN2", cfg=None):
    """Launch B/C: edge aggregation + node phase for one GAT layer.

    los/his: int arrays [nblocks, T] of band starts / max dstloc (baked into
    matmul partition offsets; his < 0 marks tiles empty on every core).
    Inputs: ea [P, nblocks*EAC*T] bf16 c-major edge rows, meta [P, nblocks*T]
    bf16 (dstlocal, 300 sentinel), losr [P, nblocks*T] bf16 (lo_t replicated),
    iotae [P, D*T] bf16 (iotae[p, d*T+t] = d), iotaw [P, W*max(T-1,1)] bf16
    (iotaw[p, i*(T-1)+t'] = i), wnext [D, wcols] bf16, bcol [P, 1] f32,
    ident [P, P] bf16.  Output: zrows [nper, wcols] f32 (no out bias).
    """
    cfg = dict(CONFIG if cfg is None else cfg)
    abl = set(cfg.get("ablate", ()))
    gw_dve_j = int(cfg.get("gw_dve_j", 32))
    band_eng = cfg.get("band_eng", "gp")
    a01f_eng = cfg.get("a01f_eng", "dve")
    use_div = bool(cfg.get("div", True))
    odt = f32 if is_last else bf16
    TB = max(T - 1, 1)

    nc = bacc.Bacc(trn_type, target_bir_lowering=False, debug=False, num_devices=NCORES)
    ea_d = nc.dram_tensor("ea", [P, nblocks * EAC * T], bf16, kind="ExternalInput")
    meta_d = nc.dram_tensor("meta", [P, nblocks * T], bf16, kind="ExternalInput")
    losr_d = nc.dram_tensor("losr", [P, nblocks * T], bf16, kind="ExternalInput")
    iotae_d = nc.dram_tensor("iotae", [P, D], bf16, kind="ExternalInput")
    iotaw_d = nc.dram_tensor("iotaw", [P, W * TB], bf16, kind="ExternalInput")
    wnext_d = nc.dram_tensor("wnext", [D, wcols], bf16, kind="ExternalInput")
    bcol_d = nc.dram_tensor("bcol", [P, 1], f32, kind="ExternalInput")
    ident_d = nc.dram_tensor("ident", [P, P], bf16, kind="ExternalInput")
    out_d = nc.dram_tensor("zrows", [nper, wcols], odt, kind="ExternalOutput")

    with tile.TileContext(nc) as tc:
        with (
            tc.tile_pool(name="const", bufs=1) as cpool,
            tc.tile_pool(name="work", bufs=int(cfg.get("wbufs", 4))) as pool,
            tc.tile_pool(name="gath", bufs=int(cfg.get("gbufs", 4))) as gpool,
            tc.tile_pool(name="a01p", bufs=int(cfg.get("abufs", 4))) as apool,
            tc.tile_pool(name="psum", bufs=2, space="PSUM") as psum,
            tc.tile_pool(name="psU", bufs=int(cfg.get("ubufs", 4)), space="PSUM") as psU,
        ):
            meta = cpool.tile([P, nblocks * T], bf16, tag="meta")
            nc.sync.dma_start(meta[:], meta_d[:])
            losr = cpool.tile([P, nblocks * T], bf16, tag="losr")
            nc.sync.dma_start(losr[:], losr_d[:])
            iotae = cpool.tile([P, D], bf16, tag="iotae")
            nc.sync.dma_start(iotae[:], iotae_d[:])
            iotaw = cpool.tile([P, W * TB], bf16, tag="iotaw")
            nc.sync.dma_start(iotaw[:], iotaw_d[:])
            wnext = cpool.tile([D, wcols], bf16, tag="wnext")
            nc.sync.dma_start(wnext[:], wnext_d[:])
            bcol = cpool.tile([P, 1], f32, tag="bcol")
            nc.sync.dma_start(bcol[:], bcol_d[:])
            ident = cpool.tile([P, P], bf16, tag="ident")
            nc.sync.dma_start(ident[:], ident_d[:])

            iotawv = iotaw[:].rearrange("p (i t) -> p i t", t=TB)

            for rep in range(repeat):
                if rep:
                    tc.strict_bb_all_engine_barrier()
                for b in range(nblocks):
                    mt = meta[:, b * T : (b + 1) * T]

                    # matmul plan: tile 0 full-width (PSUM init); banded tiles
                    # as 1-2 32-span matmuls (second only if band crosses)
                    plan = []  # (t, a, span)
                    for t in range(1, T):
                        if int(his[b, t]) < 0:
                            continue
                        a = int(los[b, t])
                        if W == P:
                            plan.append((t, 0, P))
                            continue
                        plan.append((t, a, 32))
                        if int(his[b, t]) >= a + 32:
                            plan.append((t, a + 32, 32))
                    if "mm" in abl:
                        plan = []
                    nmm = 1 + len(plan)

                    # one-hot builds depend only on constants: emit them first
                    # so DVE/Pool run ahead while the edge DMA is in flight
                    a01f = apool.tile([P, D], bf16, tag="a01f")
                    if "onehot" not in abl:
                        in0f = _ap_with(mt[:, 0:1], [mt.ap[0], [0, D]])
                        in1f = _ap_with(iotae[:], [iotae[:].ap[0], [1, D]])
                        feng = nc.vector if a01f_eng == "dve" else nc.gpsimd
                        feng.tensor_tensor(out=a01f[:], in0=in0f, in1=in1f,
                                           op=mybir.AluOpType.is_equal)
                    elif b == 0 and rep == 0:
                        nc.vector.memset(a01f[:], 0.0)
                    if T > 1 and "onehot" not in abl:
                        dl = pool.tile([P, TB], bf16, tag="dl")
                        beng = nc.gpsimd if band_eng == "gp" else nc.vector
                        beng.tensor_tensor(
                            out=dl[:], in0=mt[:, 1:T],
                            in1=losr[:, b * T + 1 : (b + 1) * T],
                            op=mybir.AluOpType.subtract,
                        )
                        AB = apool.tile([P, W * TB], bf16, tag="AB")
                        abv = AB[:].rearrange("p (i t) -> p i t", t=TB)
                        in0b = _ap_with(dl[:], [dl.ap[0], [0, W], dl.ap[1]])
                        bdi = int(cfg.get("band_dve_i", 16))
                        for eng, i0, i1 in (
                            (nc.vector, 0, bdi),
                            (nc.gpsimd, bdi, W),
                        ):
                            if i1 <= i0:
                                continue
                            in0s = _ap_with(
                                in0b, [in0b.ap[0], [0, i1 - i0], in0b.ap[2]]
                            )
                            eng.tensor_tensor(
                                out=abv[:, i0:i1, :], in0=in0s,
                                in1=iotawv[:, i0:i1, :],
                                op=mybir.AluOpType.is_equal,
                            )

                    G = gpool.tile([P, EAC * T], bf16, tag="G")
                    if "dma" not in abl:
                        nc.sync.dma_start(G[:], ea_d[:, b * EAC * T : (b + 1) * EAC * T])
                    else:
                        nc.sync.dma_start(G[:, 0:2], ea_d[:, 0:2])

                    # w = exp(lrelu(e)); lrelu = (e*0.2) max e fused on DVE so ACT
                    # only ever runs Exp/Relu/Copy (one function table)
                    ecols = G[:, D * T : EAC * T]  # [P, 4*T]
                    if "act" in abl:
                        pass
                    elif cfg.get("lrelu", "stt") == "stt":
                        nc.vector.scalar_tensor_tensor(
                            out=ecols, in0=ecols, scalar=float(NEG_SLOPE), in1=ecols,
                            op0=mybir.AluOpType.mult, op1=mybir.AluOpType.max,
                        )
                        nc.scalar.activation(ecols, ecols,
                                             mybir.ActivationFunctionType.Exp)
                    else:
                        w2 = pool.tile([P, HEADS * T], bf16, tag="w2")
                        nc.scalar.activation(w2[:], ecols,
                                             mybir.ActivationFunctionType.Exp,
                                             scale=NEG_SLOPE)
                        nc.scalar.activation(ecols, ecols,
                                             mybir.ActivationFunctionType.Exp)
                        nc.vector.tensor_tensor(out=ecols, in0=ecols, in1=w2[:],
                                                op=mybir.AluOpType.max)

                    # h *= w (head-expanded), in place, split by j-cols
                    gkjt = G[:, 0 : D * T].rearrange(
                        "p (k j t) -> p k j t", j=D // HEADS, t=T
                    )
                    wkt = G[:, D * T : EAC * T].rearrange("p (k t) -> p k t", t=T)
                    nsplit = max(1, int(cfg.get("gw_tsplit", 2)))
                    tcuts = [] if "gw" in abl else [T * i // nsplit for i in range(nsplit + 1)]
                    for tl_, tr_ in zip(tcuts[:-1], tcuts[1:]):
                        for eng, j0, j1 in (
                            (nc.vector, 0, gw_dve_j),
                            (nc.gpsimd, gw_dve_j, D // HEADS),
                        ):
                            if j1 <= j0:
                                continue
                            o = gkjt[:, :, j0:j1, tl_:tr_]
                            wk = wkt[:, :, tl_:tr_]
                            wexp = _ap_with(
                                wk, [wk.ap[0], wk.ap[1], [0, j1 - j0], wk.ap[2]]
                            )
                            eng.tensor_tensor(out=o, in0=o, in1=wexp,
                                              op=mybir.AluOpType.mult)

                    g4 = G[:].rearrange("p (c t) -> p t c", t=T)
                    U = psU.tile([P, EAC], f32, tag="U")
                    nc.tensor.matmul(U[:], lhsT=a01f[:], rhs=g4[:, 0, :],
                                     start=True, stop=(nmm == 1),
                                     tile_position=(0, 0))
                    if T > 1:
                        if "onehot" in abl:
                            AB = apool.tile([P, W * TB], bf16, tag="AB")
                            if b == 0 and rep == 0:
                                nc.vector.memset(AB[:], 0.0)
                        ab4 = AB[:].rearrange("p (i t) -> p t i", t=TB)
                        for im, (t, a, span) in enumerate(plan):
                            lo0 = int(los[b, t])
                            i0 = a - lo0
                            nc.tensor.matmul(
                                U[a : a + span, :],
                                lhsT=ab4[:, t - 1, i0 : i0 + span],
                                rhs=g4[:, t, :], start=False,
                                stop=(im == len(plan) - 1),
                                skip_group_check=True,
                                tile_position=(0, a),
                            )

                    if "node" in abl:
                        continue
                    # node phase: h0 = U[:, :D] / s (pad-node fake edges keep s>0)
                    sexp = _ap_with(U[:, D:EAC], [U[:, D:EAC].ap[0], [1, HEADS],
                                                  [0, D // HEADS]])
                    h0 = pool.tile([P, D], bf16, tag="h0")
                    h0v = h0[:].rearrange("p (k j) -> p k j", j=D // HEADS)
                    uv = U[:, 0:D].rearrange("p (k j) -> p k j", j=D // HEADS)
                    if use_div:
                        nc.vector.tensor_tensor(out=h0v, in0=uv, in1=sexp,
                                                op=mybir.AluOpType.divide)
                    else:
                        rcp = pool.tile([P, HEADS], f32, tag="rcp")
                        nc.vector.reciprocal(rcp[:], U[:, D:EAC])
                        rexp = _ap_with(rcp[:], [rcp[:].ap[0], [1, HEADS],
                                                 [0, D // HEADS]])
                        nc.vector.tensor_tensor(out=h0v, in0=uv, in1=rexp,
                                                op=mybir.AluOpType.mult)

                    # transpose; relu(hT + b) fused on ACT (same table as Exp);
                    # project; z copied out on ACT (Copy, same table)
                    hTp = psum.tile([P, P], bf16, tag="hTp")
                    nc.tensor.transpose(hTp[:], h0[:], ident[:])
                    hT = pool.tile([P, P], bf16, tag="hT")
                    nc.scalar.activation(hT[:], hTp[:], mybir.ActivationFunctionType.Relu,
                                         bias=bcol[:])
                    zp = psum.tile([P, wcols], f32, tag="zp")
                    nc.tensor.matmul(zp[:], lhsT=hT[:], rhs=wnext[:], start=True, stop=True)
                    z = pool.tile([P, wcols], odt, tag="z")
                    nc.scalar.activation(z[:], zp[:], mybir.ActivationFunctionType.Copy)
                    nc.sync.dma_start(out_d[b * P : (b + 1) * P, :], z[:])

    nc.compile()
    return nc


def prep_edges(edge_index, n, ncores):
    """Sort self-looped edges (plus fake pad-node edges) by dst; uniform-T
    t-major slot layout with per-tile dst bands.

    Returns (T, W, nblocks, slots, srcs, dsts, metas, los, losr, npad):
    slots[c][e] = padded linear index b*128*T + (j%128)*T + (j//128);
    los [nblocks, T] int band starts shared across cores; losr its [P, nb*T]
    bf16 replication; metas[c] [P, nblocks*T] bf16 dstlocal (300 = empty).
    """
    nper = -(-n // (ncores * P)) * P
    npad = nper * ncores
    nblocks = nper // P

    e0 = np.asarray(edge_index[0], dtype=np.int64)
    e1 = np.asarray(edge_index[1], dtype=np.int64)
    loops = np.arange(n, dtype=np.int64)
    # fake edges for padded nodes: enough per node to keep tile dst-bands
    # narrow AND guarantee s > 0 (no eps / NaN handling on device)
    fake_per = max(1, int(round((len(e0) + n) / max(n, 1))))
    pads = np.repeat(np.arange(n, npad, dtype=np.int64), fake_per)
    src = np.concatenate([e0, loops, np.zeros(len(pads), np.int64)])
    dst = np.concatenate([e1, loops, pads])
    order = np.argsort(dst, kind="stable")
    srcs_all = src[order]
    dsts_all = dst[order]

    bounds = np.searchsorted(dsts_all, np.arange(0, npad + 1, P))
    counts = bounds[1:] - bounds[:-1]
    T = max(1, int(-(-counts.max() // P)))

    # band bounds per (core, block, tile)
    lo_s = np.full((nblocks, T), 128, dtype=np.int64)
    hi_s = np.full((nblocks, T), -1, dtype=np.int64)
    slots, srcs, dsts, metas = [], [], [], []
    for c in range(ncores):
        meta_pad = np.full(nblocks * P * T, 300.0, dtype=np.float32)
        sl_parts = []
        lo_c, hi_c = int(bounds[c * nblocks]), int(bounds[(c + 1) * nblocks])
        for b in range(nblocks):
            g = c * nblocks + b
            lo, hi = int(bounds[g]), int(bounds[g + 1])
            cnt = hi - lo
            j = np.arange(cnt)
            slot = b * P * T + (j % P) * T + (j // P)
            sl_parts.append(slot)
            dloc = (dsts_all[lo:hi] - g * P).astype(np.int64)
            meta_pad[slot] = dloc.astype(np.float32)
            ntile = -(-cnt // P)
            for t in range(1, ntile):  # tile 0 is always full-width
                seg = dloc[t * P : (t + 1) * P]
                lo_s[b, t] = min(lo_s[b, t], int(seg[0]))
                hi_s[b, t] = max(hi_s[b, t], int(seg[-1]))
        slots.append(np.concatenate(sl_parts))
        srcs.append(srcs_all[lo_c:hi_c])
        dsts.append(dsts_all[lo_c:hi_c])
        meta = (
            meta_pad.reshape(nblocks, P, T).transpose(1, 0, 2).reshape(P, nblocks * T)
        )
        metas.append(np.ascontiguousarray(meta.astype(nbf16)))

    # PE PSUM writes are power-of-2 tiled: span<=32 may start at 0/32/64/96.
    # Band = 64-wide window at a 32-aligned start (covers any raw band <= 33
    # wide); aggregation emits 1-2 32-span matmuls per tile.
    a_s = np.where(hi_s >= 0, np.minimum((lo_s // 32) * 32, P - P // 2), 0)
    if bool(((hi_s < a_s + P // 2) | (hi_s < 0)).all()):
        W = P // 2
    else:  # fallback: full-width one-hot for every tile
        W = P
        a_s = np.zeros_like(a_s)
    losr = np.ascontiguousarray(
        np.broadcast_to(
            a_s.reshape(1, nblocks * T).astype(np.float32), (P, nblocks * T)
        ).astype(nbf16)
    )
    return T, W, nblocks, slots, srcs, dsts, metas, a_s, hi_s, losr, npad


def build_ea(nrows_full, slots, srcs, dsts, nblocks, T):
    """EA[p, b*132T + c*T + t] = col c of [h[src] | a_src[src]+a_dst[dst]]."""
    eas = []
    for sl, ss, ds in zip(slots, srcs, dsts):
        vals = np.zeros((len(ss), EAC), dtype=nbf16)
        vals[:, 0:D] = nrows_full[ss, 0:D]
        vals[:, D:EAC] = (
            nrows_full[ss, D : D + HEADS].astype(np.float32)
            + nrows_full[ds, D + HEADS : ROW].astype(np.float32)
        ).astype(nbf16)
        pad = np.zeros((nblocks * P * T, EAC), dtype=nbf16)
        pad[sl] = vals
        ea = (
            pad.reshape(nblocks, P, T, EAC)
            .transpose(1, 0, 3, 2)
            .reshape(P, nblocks * EAC * T)
        )
        eas.append(np.ascontiguousarray(ea))
    return eas


def amat(att):
    A = np.zeros((D, HEADS), dtype=np.float32)
    att = np.asarray(att, dtype=np.float32)
    for h in range(HEADS):
        A[h * (D // HEADS) : (h + 1) * (D // HEADS), h] = att[h]
    return A


_cache = {}


def run_gat(x, edge_index, W1, att_src1, att_dst1, b1, W2, att_src2, att_dst2, b2,
            Wc, bc, n=None, ncores=NCORES, repeat=1):
    global LAST_INFO
    x = np.asarray(x, dtype=np.float32)
    if n is None:
        n = int(x.shape[0])

    t0 = time.time()
    T, W, nblocks, slots, srcs, dsts, metas, los, his, losr, npad = prep_edges(
        edge_index, n, ncores
    )
    nper = npad // ncores
    cfg = dict(CONFIG)
    key = (npad, T, W, nblocks, los.tobytes(), his.tobytes(), ncores, repeat,
           tuple(sorted(cfg.items())))
    t1 = time.time()
    if key in _cache:
        ncA, ncB, ncC = _cache[key]
    else:
        ncA = build_node_transform(nper, ROW)
        ncB = build_gat_layer(nper, nblocks, T, W, los, his, ROW, is_last=False,
                              repeat=repeat, cfg=cfg)
        ncC = build_gat_layer(nper, nblocks, T, W, los, his, C, is_last=True,
                              repeat=repeat, cfg=cfg)
        _cache[key] = (ncA, ncB, ncC)
    t2 = time.time()

    W1 = np.asarray(W1, dtype=np.float32)
    W2 = np.asarray(W2, dtype=np.float32)
    Wc = np.asarray(Wc, dtype=np.float32)
    w1cat = np.concatenate([W1, W1 @ amat(att_src1), W1 @ amat(att_dst1)], axis=1)
    w2cat = np.concatenate([W2, W2 @ amat(att_src2), W2 @ amat(att_dst2)], axis=1)
    b1c = np.asarray(b1, np.float32).reshape(P, 1)
    b2c = np.asarray(b2, np.float32).reshape(P, 1)
    bc = np.asarray(bc, dtype=np.float32)
    TB = max(T - 1, 1)
    iotae = np.ascontiguousarray(
        np.broadcast_to(np.arange(P, dtype=np.float32)[None, :], (P, P)).astype(nbf16)
    )
    iotaw = np.ascontiguousarray(
        np.broadcast_to(
            np.repeat(np.arange(W, dtype=np.float32), TB)[None, :], (P, W * TB)
        ).astype(nbf16)
    )
    ident = np.eye(P, dtype=nbf16)

    xp = np.zeros((npad, D), dtype=np.float32)
    xp[:n] = x

    # Launch A
    mapsA = [
        {
            "xts": np.ascontiguousarray(xp[c * nper : (c + 1) * nper].T).astype(nbf16),
            "wcat": w1cat.astype(nbf16),
        }
        for c in range(ncores)
    ]
    resA = run_bass_kernel_spmd(ncA, mapsA, list(range(ncores)))
    nrows_full = np.concatenate([resA.results[c]["nrows"] for c in range(ncores)], axis=0)
    t3 = time.time()

    # Expansion 1 + Launch B
    eas = build_ea(nrows_full, slots, srcs, dsts, nblocks, T)
    mapsB = [
        {
            "ea": eas[c], "meta": metas[c], "losr": losr, "iotae": iotae,
            "iotaw": iotaw, "wnext": w2cat.astype(nbf16), "bcol": b1c,
            "ident": ident,
        }
        for c in range(ncores)
    ]
    resB = run_bass_kernel_spmd(ncB, mapsB, list(range(ncores)))
    zrows_full = np.concatenate([resB.results[c]["zrows"] for c in range(ncores)], axis=0)
    t4 = time.time()

    # Expansion 2 + Launch C
    eas2 = build_ea(zrows_full, slots, srcs, dsts, nblocks, T)
    mapsC = [
        {
            "ea": eas2[c], "meta": metas[c], "losr": losr, "iotae": iotae,
            "iotaw": iotaw, "wnext": Wc.astype(nbf16), "bcol": b2c,
            "ident": ident,
        }
        for c in range(ncores)
    ]
    resC = run_bass_kernel_spmd(ncC, mapsC, list(range(ncores)))
    out = np.concatenate([resC.results[c]["zrows"] for c in range(ncores)], axis=0)[:n]
    out = out + bc[None, :]
    t5 = time.time()

    LAST_INFO = {
        "prep_s": t1 - t0, "build_s": t2 - t1, "launchA_s": t3 - t2,
        "launchB_s": t4 - t3, "launchC_s": t5 - t4,
        "ncs": (ncA, ncB, ncC),
        "maps": (mapsA, mapsB, mapsC),
        "dims": (T, W, nblocks, nper, los, his),
    }
    print(
        f"[kernel] prep={t1 - t0:.2f}s build={t2 - t1:.2f}s A={t3 - t2:.2f}s "
        f"B={t4 - t3:.2f}s C={t5 - t4:.2f}s T={T} W={W}",
        file=sys.stderr,
    )
    return out.astype(np.float32)


def kernel(x, edge_index, W1, att_src1, att_dst1, b1, W2, att_src2, att_dst2, b2, Wc, bc):
    return run_gat(x, edge_index, W1, att_src1, att_dst1, b1,
                   W2, att_src2, att_dst2, b2, Wc, bc)
